# revision 22
# baseline (speedup 1.0000x reference)
"""MinGRU layer kernel for 8 Trainium2 NeuronCores.

Problem: x (4, 8192, 1024) f32; Wz, Wh (1024, 1024); bz, bh (1024,)
    z = sigmoid(x @ Wz + bz); h_tilde = x @ Wh + bh
    h_t = (1 - z_t) * h_{t-1} + z_t * h_tilde_t   (scan over seq, h_{-1} = 0)

Sharding: 8 cores = 4 batches x 2 output-dim halves. The scan is
independent per (batch, dim), so each core owns a full-sequence scan for
one batch and 512 of the 1024 output dims -- no cross-core traffic.

Precision: the z-path matmul runs in fp8 e4m3 DoubleRow mode (2x PE
throughput; K=256 per instruction). Wz is pre-scaled by 32 on the host so
its values sit in e4m3's normal range; the ACT sigmoid compensates with
scale=1/32. The h-path matmul stays fp16 -- fp8 there would push the L2
error (4.1e-2) over the 2e-2 gate, while z-only-fp8 measures 1.74e-2.
PE floor: 109.3us (h, fp16) + 54.6us (z, fp8 2x) = 164us vs 218.6us
all-fp16.

DMA: the HWDGE hardware queue costs ~4.3ns per line (contiguous run), so
the host packs x per (chunk, partition) -- each chunk transfer is 128
lines of 8*chunk elems instead of 1024 short lines. Same for the output:
the four m-tiles share one [128, 4*chunk] fp16 tile stored as one
contiguous 128-line DMA per chunk (host unpacks + upcasts). Startup
transfers are interleaved on the sync ring in PE consumption order
(wh k0-3 -> x16 c0 -> wz8 -> x8 c0 -> wh k4-7); stores ride the scalar
ring's queue; SWDGE (gpsimd, ~90GB/s) is never used for data. Each
dma_start costs ~0.65us of issuing-engine time, so bias+Wh travel as one
byte-packed tensor read back through bitcast views.

Measured on HW: 189.6us (baseline 244.0us). PE busy ~170us vs the
164-168us fp16+fp8 matmul floor at the ~2.35GHz sustained clock; ~7.5us
fixed NEFF preamble + ~2.5us exit barriers; DVE scan+STT ~133us and ACT
~100us stay off the critical path.
"""

import sys

if "/opt/trn_rl_repo" not in sys.path:
    sys.path.insert(0, "/opt/trn_rl_repo")

import numpy as np

from concourse import bass, mybir
from concourse.tile import TileContext
from concourse.bass_utils import run_bass_kernel_spmd

BATCH, SEQ, D = 4, 8192, 1024
DH = 512            # output dims per core
N_CORES = 8
# Seq chunk schedule: small chunks first so the PE starts on real work
# early (warms the HAM clock gate) and the consumer engines ramp before
# the PE hits full streaming rate.
CHUNKS = [256, 256, 512] + [1024] * 6 + [512, 256, 128, 128]
assert sum(CHUNKS) == SEQ
CHUNK_MAX = max(CHUNKS)
NM = DH // 128      # output-dim tiles per core
NK = D // 128      # contraction tiles (fp16 h-path)
NK2 = D // 256      # DoubleRow contraction tiles (fp8 z-path)

F8 = mybir.dt.float8e4
F16 = mybir.dt.float16
F32 = mybir.dt.float32
AF = mybir.ActivationFunctionType
OP = mybir.AluOpType
DR = mybir.MatmulPerfMode.DoubleRow

WZ_SCALE = 32.0     # host multiplies Wz by this before the e4m3 cast


_WAIT_LIMIT = 1  # this walrus build rejects multiple sem waits per instruction


def _split_sync_waits(nc):
    """Move excess semaphore waits (beyond _WAIT_LIMIT) off each instruction
    onto same-engine nops inserted immediately before it. Waits only gate
    execution, so hoisting some onto a preceding nop in the same engine
    stream is semantics-preserving."""
    import bass_rust

    n_extra = 0
    for fn in nc.m.functions:
        for blk in fn.blocks:
            insts = blk.instructions
            out = []
            for inst in insts:
                si = inst.sync_info
                if si is not None and si.on_wait and len(si.on_wait) > _WAIT_LIMIT:
                    waits = list(si.on_wait)
                    head, tail = waits[:-_WAIT_LIMIT], waits[-_WAIT_LIMIT:]
                    for j in range(0, len(head), _WAIT_LIMIT):
                        n_extra += 1
                        nop = bass_rust.InstNoOp(
                            name=f"{inst.name}-waitsplit{j}",
                            engine=inst.engine,
                            sync_info=type(si)(
                                on_wait=head[j:j + _WAIT_LIMIT], on_update=[]
                            ),
                            bass_nofuse=True,
                        )
                        nc.register_instruction(nop, overwrite=True)
                        out.append(nop)
                    si.on_wait = tail
                out.append(inst)
            if n_extra:
                blk.instructions = out
    return n_extra


def _build_program(chunks=CHUNKS):
    seq = sum(chunks)
    nchunk = len(chunks)
    chunk_max = max(chunks)

    nc = bass.Bass("TRN2", target_bir_lowering=False, debug=False)

    # Host-packed layouts (see _make_in_maps):
    #   xP16/xP8[p, 8*t_off + i*chunk + t] = x[seq_off+t, i*128+p] per chunk
    #   wz8p[p, (2*kt+i)*DH + m]         = Wz[256*kt + 128*i + p, m] * 32
    #   hTp[p, 4*seq_off + m*chunk + t]  = h[seq_off+t, m*128+p]
    xP16 = nc.dram_tensor("xP16", [128, NK * seq], F16, kind="ExternalInput").ap()
    xP8 = nc.dram_tensor("xP8", [128, NK * seq], F8, kind="ExternalInput").ap()
    wz8p = nc.dram_tensor("wz8p", [128, NK * DH // 128 * 128], F8,
                          kind="ExternalInput").ap()
    # bias + Wh in one byte-packed tensor -> a single DMA issue (each
    # dma_start costs ~0.65us of issuing-engine time; 10 separate weight
    # issues delayed wz8's data to 24us and stalled the PE 3.7us).
    # Per partition p: [bias row p: 48B f32 | wh rows kt*128+p: 1KB fp16 x8]
    WPACK_BYTES = 4 * 3 * NM + 2 * DH * NK
    wpack = nc.dram_tensor("wpack", [128, WPACK_BYTES], mybir.dt.uint8,
                           kind="ExternalInput").ap()
    hTp = nc.dram_tensor("hTp", [128, NM * seq], F16, kind="ExternalOutput").ap()

    with TileContext(nc) as tc:
        with (
            tc.tile_pool(name="weights", bufs=1) as wpool,
            tc.tile_pool(name="bias", bufs=1) as biaspool,
            tc.tile_pool(name="xt", bufs=4) as xpool,
            tc.tile_pool(name="x8t", bufs=4) as x8pool,
            tc.tile_pool(name="a", bufs=4) as apool,
            tc.tile_pool(name="z", bufs=4) as zpool,
            tc.tile_pool(name="b", bufs=4) as bpool,
            tc.tile_pool(name="h", bufs=4) as hpool,
            tc.tile_pool(name="psz", bufs=4, space="PSUM") as pszpool,
            tc.tile_pool(name="psh", bufs=4, space="PSUM") as pshpool,
        ):
            # bias + Wh in two DMAs/tiles on the scalar HWDGE ring
            # ([bias+k0-3], [k4-7]) -- the PE starts on the first half ~2us
            # sooner than with a single 1MB transfer, and k4-7 land just as
            # k0-3 are consumed. bitcast views carve out the typed tiles.
            # wz8 third.
            # Startup transfers interleaved on the sync ring (q1 -- it wakes
            # ~1.6us before the scalar ring's queue) in exactly PE
            # consumption order: each tranche of weights/x lands as the PE
            # finishes the previous one, so the ramp has no stalls:
            #   wpkA (bias+wh k0-3) -> x16 c0 -> wz8 -> x8 c0 -> wpkB (k4-7)
            # Split the cold-start bytes across BOTH hardware queues so the
            # two gating transfers (wh k0-3 and x16 c0) arrive in parallel.
            half_b = 4 * 3 * NM + 2 * DH * (NK // 2)
            wpkA = wpool.tile([128, half_b], mybir.dt.uint8, tag="wpackA")
            nc.sync.dma_start(out=wpkA[:], in_=wpack[:, :half_b])
            chunk0 = chunks[0]
            xt0 = xpool.tile([128, NK * chunk_max], F16, tag="x16")
            nc.scalar.dma_start(out=xt0[:, :NK * chunk0],
                                in_=xP16[:, :NK * chunk0])
            wz8_t = wpool.tile([128, 2 * NK2, DH], F8, tag="wz8")
            nc.sync.dma_start(out=wz8_t[:], in_=wz8p[:])
            x8t0 = x8pool.tile([128, NK * chunk_max], F8, tag="x8")
            nc.sync.dma_start(out=x8t0[:, :NK * chunk0],
                              in_=xP8[:, :NK * chunk0])
            wpkB = wpool.tile([128, WPACK_BYTES - half_b], mybir.dt.uint8,
                              tag="wpackB")
            nc.scalar.dma_start(out=wpkB[:], in_=wpack[:, half_b:])
            bias_t = wpkA[:, 0:4 * 3 * NM].bitcast(F32)
            bz_t = [bias_t[:, m:m + 1] for m in range(NM)]
            bzn_t = [bias_t[:, NM + m:NM + m + 1] for m in range(NM)]
            bh_t = [bias_t[:, 2 * NM + m:2 * NM + m + 1] for m in range(NM)]
            wh_o = 4 * 3 * NM
            wh_b = [
                (wpkA[:, wh_o + 2 * DH * kt: wh_o + 2 * DH * (kt + 1)]
                 if kt < NK // 2 else
                 wpkB[:, 2 * DH * (kt - NK // 2): 2 * DH * (kt - NK // 2 + 1)]
                 ).bitcast(F16)
                for kt in range(NK)
            ]
            wh_t = [[wh_b[kt][:, m * 128:(m + 1) * 128] for m in range(NM)]
                    for kt in range(NK)]

            last_h = [None] * NM
            seq_off = 0
            for c in range(nchunk):
                chunk = chunks[c]
                # One packed DMA per chunk per precision: 128 contiguous
                # lines of 8*chunk elems each. (chunk 0 was prefetched
                # interleaved with the weight loads above.)
                if c == 0:
                    xt, x8t = xt0, x8t0
                else:
                    xt = xpool.tile([128, NK * chunk_max], F16, tag="x16")
                    nc.sync.dma_start(
                        out=xt[:, :NK * chunk],
                        in_=xP16[:, NK * seq_off:NK * (seq_off + chunk)],
                    )
                    x8t = x8pool.tile([128, NK * chunk_max], F8, tag="x8")
                    nc.sync.dma_start(
                        out=x8t[:, :NK * chunk],
                        in_=xP8[:, NK * seq_off:NK * (seq_off + chunk)],
                    )
                xv = xt[:, :NK * chunk].rearrange("p (i t) -> p i t", i=NK)
                xv8 = x8t[:, :NK * chunk].rearrange("p (i t) -> p i t", i=NK)

                # All 4 m-tiles share one fp16 h tile -> one packed
                # contiguous store per chunk.
                h_t3 = hpool.tile([128, NM * chunk_max], F16, tag="h")
                hv = h_t3[:, :NM * chunk].rearrange("p (m t) -> p m t", m=NM)

                bounds = []
                acc = 0
                while acc < chunk:
                    bounds.append((acc, min(chunk, acc + 512)))
                    acc = min(chunk, acc + 512)
                for w0, w1 in bounds:
                    for m in range(NM):
                        psz = pszpool.tile([128, 512], F32)
                        psh = pshpool.tile([128, 512], F32)
                        # z-path: fp8 DoubleRow, K=256 per matmul, 2x rate
                        for kt in range(NK2):
                            nc.tensor.matmul(
                                psz[:, :w1 - w0],
                                wz8_t[:, 2 * kt:2 * kt + 2, m * 128:(m + 1) * 128],
                                xv8[:, 2 * kt:2 * kt + 2, w0:w1],
                                start=(kt == 0),
                                stop=(kt == NK2 - 1),
                                perf_mode=DR,
                            )
                        # h-path: fp16
                        for kt in range(NK):
                            nc.tensor.matmul(
                                psh[:, :w1 - w0],
                                wh_t[kt][m],
                                xv[:, kt, w0:w1],
                                start=(kt == 0),
                                stop=(kt == NK - 1),
                            )
                        # z first: the DVE multiply consumes it, so z-then-a
                        # shortens the STT->scan critical path by one ACT op.
                        # psz holds 32*z_pre (Wz host-scaled); ACT scale
                        # compensates.
                        z_t = zpool.tile([128, 512], F32)
                        nc.scalar.activation(z_t[:, :w1 - w0], psz[:, :w1 - w0],
                                             AF.Sigmoid,
                                             bias=bz_t[m][:], scale=1.0 / WZ_SCALE)
                        # a = 1 - sigmoid(z_pre + bz) = sigmoid(-z_pre - bz)
                        a_t = apool.tile([128, 512], F16)
                        if c == nchunk - 1:
                            # Tail: the 8 ACT sigmoids serialize after the
                            # last matmul; computing a = 1 - z on the idle
                            # GpSimd halves that chain.
                            nc.gpsimd.tensor_scalar(
                                a_t[:, :w1 - w0], z_t[:, :w1 - w0],
                                -1.0, 1.0, op0=OP.mult, op1=OP.add)
                        else:
                            nc.scalar.activation(a_t[:, :w1 - w0],
                                                 psz[:, :w1 - w0],
                                                 AF.Sigmoid,
                                                 bias=bzn_t[m][:],
                                                 scale=-1.0 / WZ_SCALE)
                        # b = (h_pre + bh) * z
                        b_t = bpool.tile([128, 512], F16)
                        nc.vector.scalar_tensor_tensor(
                            b_t[:, :w1 - w0], psh[:, :w1 - w0], bh_t[m][:],
                            z_t[:, :w1 - w0],
                            op0=OP.add, op1=OP.mult,
                        )
                        # h_t = a_t * h_{t-1} + b_t along seq (fp16 out)
                        h_t = hv[:, m, w0:w1]
                        init = 0.0 if last_h[m] is None else last_h[m][:, -1:]
                        nc.vector.tensor_tensor_scan(
                            h_t, a_t[:, :w1 - w0], b_t[:, :w1 - w0], init,
                            op0=OP.mult, op1=OP.add,
                        )
                        last_h[m] = h_t
                # One packed store per chunk on the scalar HWDGE ring; the
                # final chunk splits across sync+scalar so the flush
                # overlaps the last scans.
                if c == nchunk - 1:
                    # Per-m stores so each flushes right after its scan.
                    tail_eng = [nc.sync, nc.scalar, nc.sync, nc.scalar]
                    for mm in range(NM):
                        tail_eng[mm].dma_start(
                            out=hTp[:, NM * seq_off + mm * chunk:
                                    NM * seq_off + (mm + 1) * chunk],
                            in_=h_t3[:, mm * chunk:(mm + 1) * chunk],
                        )
                else:
                    nc.scalar.dma_start(
                        out=hTp[:, NM * seq_off:NM * (seq_off + chunk)],
                        in_=h_t3[:, :NM * chunk],
                    )
                seq_off += chunk
    _split_sync_waits(nc)
    return nc


_NC_CACHE = None


def _get_program():
    global _NC_CACHE
    if _NC_CACHE is None:
        _NC_CACHE = _build_program()
    return _NC_CACHE


def _pack_x(xb, np_dtype, chunks):
    """xb: (SEQ, D) one batch. Returns [128, 8*SEQ] packed per chunk:
    row p, chunk c: [x[c0:c1, i*128+p] for i in 0..7] concatenated."""
    seq = xb.shape[0]
    out = np.empty((128, NK * seq), dtype=np_dtype)
    off = 0
    for c in chunks:
        blk = xb[off:off + c, :].astype(np_dtype)         # (c, 1024)
        # (c, 8, 128) -> (128, 8, c)
        blk = blk.reshape(c, NK, 128).transpose(2, 1, 0)
        out[:, NK * off:NK * (off + c)] = blk.reshape(128, NK * c)
        off += c
    return np.ascontiguousarray(out)


def _make_wpack(Wh_half, bz_half, bh_half):
    """[128, 48 + 8192] uint8: per partition p, bias row p (f32) then wh
    rows kt*128+p (fp16) for kt in 0..7."""
    bzc = bz_half.astype(np.float32).reshape(NM, 128).T
    bhc = bh_half.astype(np.float32).reshape(NM, 128).T
    bias = np.ascontiguousarray(np.hstack([bzc, -bzc, bhc]))  # (128, 12) f32
    wh16 = Wh_half.astype(np.float16)                          # (1024, 512)
    whp = wh16.reshape(NK, 128, DH).transpose(1, 0, 2).reshape(128, NK * DH)
    return np.ascontiguousarray(np.concatenate(
        [bias.view(np.uint8).reshape(128, -1),
         np.ascontiguousarray(whp).view(np.uint8).reshape(128, -1)], axis=1))


def _make_in_maps(x, Wz, bz, Wh, bh):
    import ml_dtypes

    f8np = ml_dtypes.float8_e4m3
    xP16 = [_pack_x(x[b], np.float16, CHUNKS) for b in range(BATCH)]
    xP8 = [_pack_x(x[b], f8np, CHUNKS) for b in range(BATCH)]
    wz8p = []
    for c in range(2):
        w = (Wz[:, c * DH:(c + 1) * DH] * WZ_SCALE).astype(f8np)  # (1024, 512)
        # row 256*kt + 128*i + p -> [p, (2*kt+i)*DH + m]
        w = w.reshape(NK2, 2, 128, DH).transpose(2, 0, 1, 3).reshape(128, -1)
        wz8p.append(np.ascontiguousarray(w))
    wpacks = [_make_wpack(Wh[:, c * DH:(c + 1) * DH],
                          bz[c * DH:(c + 1) * DH], bh[c * DH:(c + 1) * DH])
              for c in range(2)]
    in_maps = []
    for i in range(N_CORES):
        b, c = i // 2, i % 2
        in_maps.append({
            "xP16": xP16[b], "xP8": xP8[b], "wz8p": wz8p[c],
            "wpack": wpacks[c],
        })
    return in_maps


def _unpack_h(hTp, chunks):
    """hTp: [128, 4*SEQ] fp16 packed -> (SEQ, 512) f32."""
    seq = sum(chunks)
    out = np.empty((seq, DH), dtype=np.float32)
    off = 0
    for c in chunks:
        blk = hTp[:, NM * off:NM * (off + c)].astype(np.float32)
        # (128, 4, c) -> (c, 4, 128) -> (c, 512)
        blk = blk.reshape(128, NM, c).transpose(2, 1, 0).reshape(c, DH)
        out[off:off + c] = blk
        off += c
    return out


def _run(x, Wz, bz, Wh, bh, trace=False, trace_cores=None):
    import time

    nc = _get_program()
    in_maps = _make_in_maps(x, Wz, bz, Wh, bh)
    res = None
    for attempt in range(3):
        try:
            res = run_bass_kernel_spmd(
                nc, in_maps, list(range(N_CORES)),
                trace=trace, trace_cores=trace_cores,
            )
            break
        except Exception:
            # Transient NRT device errors have been observed on the first
            # execution after a fresh compile; retry.
            if attempt == 2:
                raise
            time.sleep(10)
    out = np.empty((BATCH, SEQ, D), dtype=np.float32)
    for i in range(N_CORES):
        b, c = i // 2, i % 2
        out[b, :, c * DH:(c + 1) * DH] = _unpack_h(res.results[i]["hTp"], CHUNKS)
    return out, res


def kernel(x, Wz, bz, Wh, bh):
    x = np.asarray(x, dtype=np.float32)
    Wz = np.asarray(Wz, dtype=np.float32)
    Wh = np.asarray(Wh, dtype=np.float32)
    bz = np.asarray(bz, dtype=np.float32)
    bh = np.asarray(bh, dtype=np.float32)
    out, _ = _run(x, Wz, bz, Wh, bh, trace=False)
    return out
